# revision 41
# baseline (speedup 1.0000x reference)
"""Distributed dot-product attention for TRN2, 8 NeuronCores.

Sharding: 8 cores = 4 batches x 2 head-groups (8 heads each).
Each core computes, for its (batch b, head-group g):
    Q = Xq[b] @ (Wq[g]/8).T ; K = Xk[b] @ Wk[g].T ; V = Xv[b] @ Wv[g].T
    per head h: A = exp(Q_h K_h^T); O_h = (A V_h) / rowsum(A)
    partial[b,g] = concat_h(O_h) @ Wc[:, g].T            (row-parallel)
Host: out[b] = partial[b,0] + partial[b,1] + bc          (all-reduce + bias)

Device-side layouts avoid every transpose: the host ships X^T and W^T in
bf16; S is computed transposed ([Lk, Lq]) so exp(S^T) feeds AV directly as
the moving operand; AV's stationary V carries a ones column so the softmax
denominators fall out of the same accumulation; composition consumes O^T
HEAD-PAIR blocks [128, m] (head 2p on partitions 0:64, head 2p+1 on
64:128 via a DVE quadrant-crossing write) so each comp matmul contracts
the full 128 rows; it emits the natural-layout f32 partial.

Softmax denominators: mid-kernel pairs bounce the evicted d-row through
DRAM (reshaped to [64, 16]) for a full-lane reciprocal - zero PE/ACT
involvement, deferred ~2 pairs so it hides completely; only the final
pair (the critical path into the last composition) broadcasts the d-row
with a ones-stationary K=1 matmul + reciprocal_approx_fast instead.

Schedule: the exp chain is the critical resource (ACT is the only engine
with the activation LUT; 33.5M exps/core ~ 280us busy). Input rounds load
as single 1MB DMAs in first-need order (per-trigger SWDGE cost otherwise
caps input bandwidth at ~200GB/s), only K pair 0 + Q round 0 gate the
first S^T, and all other projections / previous-quarter composition are
emitted as fine-grained fillers INSIDE the attention chunk loops so the
Tile list-scheduler drops them into PE slack under the exp-bound quarters
instead of stalling the exp chain at pair/quarter boundaries.
"""

import math
from contextlib import ExitStack

import numpy as np
import ml_dtypes

import concourse.bass as bass
import concourse.bacc as bacc
import concourse.tile as tile
from concourse import mybir
from concourse.bass_utils import run_bass_kernel_spmd

B, L, D, H = 4, 2048, 1024, 16
DH = D // H          # 64 per-head dim
HPC = H // 2         # 8 heads per core
G = HPC * DH         # 512 head-group width
N_CORES = 8

f32 = mybir.dt.float32
bf16 = mybir.dt.bfloat16


def build_nc(seq=L, debug=False):
    """Build the per-core Bass program (SPMD, identical on all cores)."""
    KD = D // 128        # 8 contraction chunks over model dim
    LK = seq // 128      # Lk chunks (16)
    NPAIR = HPC // 2     # 4 head-pairs
    NQ = seq // 512      # Lq quarters (4)

    nc = bacc.Bacc(None, target_bir_lowering=False, debug=False)

    # X^T inputs arrive as per-round contiguous 1MB blocks
    # [n, 128, KD*512] (k-major columns): one contiguous DMA per
    # 512-column round (per-trigger SWDGE cost makes eight 128KB loads
    # ~4x slower to issue than one 1MB load).
    xqT = nc.dram_tensor("xqT", [NQ, 128, KD * 512], bf16, kind="ExternalInput")
    xkT = nc.dram_tensor("xkT", [NQ, 128, KD * 512], bf16, kind="ExternalInput")
    xvT = nc.dram_tensor("xvT", [NQ, 128, KD * 512], bf16, kind="ExternalInput")
    wqT = nc.dram_tensor("wqT", [128, KD * G], bf16, kind="ExternalInput")
    wkT = nc.dram_tensor("wkT", [128, KD * G], bf16, kind="ExternalInput")
    wvT = nc.dram_tensor("wvT", [128, KD * G], bf16, kind="ExternalInput")
    wcT = nc.dram_tensor("wcT", [128, (G // 128) * D], bf16,
                         kind="ExternalInput")
    outp = nc.dram_tensor("outp", [seq, D], f32, kind="ExternalOutput")
    dbg = {}
    if debug:
        for nm, shp in [("qt", [4, 128, seq]), ("kt", [4, 128, seq]),
                        ("vt", [16, 128, 8, 65]), ("ot", [4, 128, seq]),
                        ("oc", [16, DH + 1, 1024])]:
            dbg[nm] = nc.dram_tensor(f"dbg_{nm}", shp, f32 if nm == "oc" else bf16,
                                     kind="ExternalOutput")

    with tile.TileContext(nc) as tc, ExitStack() as ctx:
        Exp = mybir.ActivationFunctionType.Exp

        # Persistent SBUF: projected Q^T/K^T (pair tiles: head 2p on
        # partitions 0:64, head 2p+1 on 64:128), V with ones column,
        # normalized O^T as head-pair tiles, composition weight pairs.
        const = ctx.enter_context(tc.tile_pool(name="const", bufs=1))
        QT_t = [const.tile([128, seq], bf16, tag=f"qt{p}", name=f"qt{p}")
                for p in range(NPAIR)]
        KT_t = [const.tile([128, seq], bf16, tag=f"kt{p}", name=f"kt{p}")
                for p in range(NPAIR)]
        V_t = [const.tile([128, HPC, DH + 1], bf16, tag=f"v{m}", name=f"v{m}")
               for m in range(LK)]
        OTP_t = [const.tile([128, seq], bf16, tag=f"otp{p}", name=f"otp{p}")
                 for p in range(NPAIR)]
        wc_all = const.tile([128, NPAIR * D], bf16, tag="wc", name="wc")
        wcP_t = [wc_all[:, p * D:(p + 1) * D] for p in range(NPAIR)]
        ones_c = const.tile([128, DH], f32, tag="ones", name="ones")
        nc.vector.memset(ones_c[:], 1.0)

        wpool = ctx.enter_context(tc.tile_pool(name="wpool", bufs=1))

        def load_w(src, pfx):
            w_all = wpool.tile([128, KD * G], bf16, tag=pfx, name=pfx, bufs=1)
            nc.gpsimd.dma_start(out=w_all[:], in_=src[:])
            return [w_all[:, k * G:(k + 1) * G] for k in range(KD)]

        xcol = ctx.enter_context(tc.tile_pool(name="xcol", bufs=2))

        def load_xround(tag, src, n, bufs=None):
            """One 1MB DMA: round n of X^T -> [128, KD*512] (k-major cols)."""
            t = xcol.tile([128, KD * 512], bf16, tag=tag, name=f"{tag}_{n}",
                          bufs=bufs)
            nc.gpsimd.dma_start(out=t[:], in_=src[n])
            return [t[:, k * 512:(k + 1) * 512] for k in range(KD)]

        # PSUM: projections/composition/denominator-broadcast pp tiles
        # (2 banks), S^T pair tiles (4 banks), AV accumulator (2) = 8.
        gen_ps = ctx.enter_context(
            tc.tile_pool(name="gen_ps", bufs=2, space=bass.MemorySpace.PSUM))
        stp_p = ctx.enter_context(
            tc.tile_pool(name="stp", bufs=2, space=bass.MemorySpace.PSUM))
        oap = ctx.enter_context(
            tc.tile_pool(name="oap", bufs=1, space=bass.MemorySpace.PSUM))
        apool = ctx.enter_context(tc.tile_pool(name="apool", bufs=2))
        nrm = ctx.enter_context(tc.tile_pool(name="nrm", bufs=2))
        ost = ctx.enter_context(tc.tile_pool(name="ost", bufs=3))
        dscr = ctx.enter_context(
            tc.tile_pool(name="dscr", bufs=2, space=bass.MemorySpace.DRAM))

        def proj_qk_round(n, w_t, x_n, dst, p):
            """dst[p][:, n-cols] = (W block p).T @ X^T[:, n-cols], bf16."""
            ps = gen_ps.tile([128, 512], f32, tag="pp", name="pp")
            for k in range(KD):
                nc.tensor.matmul(
                    ps[:], lhsT=w_t[k][:, p * 128:(p + 1) * 128],
                    rhs=x_n[k], start=(k == 0), stop=(k == KD - 1))
            nc.vector.tensor_copy(dst[p][:, n * 512:(n + 1) * 512], ps[:])

        def proj_v_chunk(m, xv_n, wv_t):
            """V_t[m] (one 128-row chunk of V, with ones column)."""
            mm = m % 4
            ps = gen_ps.tile([128, G], f32, tag="pp", name="pp")
            for k in range(KD):
                nc.tensor.matmul(
                    ps[:], lhsT=xv_n[k][:, mm * 128:(mm + 1) * 128],
                    rhs=wv_t[k], start=(k == 0), stop=(k == KD - 1))
            nc.vector.tensor_copy(
                V_t[m][:, :, 0:DH], ps[:].rearrange("p (h d) -> p h d", h=HPC))
            nc.vector.memset(V_t[m][:, :, DH:DH + 1], 1.0)

        def comp_half(m, half, o_sb):
            """One half of composition output row-chunk m."""
            msl = slice(m * 128, (m + 1) * 128)
            hsl = slice(half * 512, (half + 1) * 512)
            ps = gen_ps.tile([128, 512], f32, tag="pp", name="pp")
            for p in range(NPAIR):
                nc.tensor.matmul(
                    ps[:], lhsT=OTP_t[p][:, msl], rhs=wcP_t[p][:, hsl],
                    start=(p == 0), stop=(p == NPAIR - 1))
            nc.vector.tensor_copy(o_sb[:, hsl], ps[:])

        def attention_quarter(p, q, hook=None):
            qsl = slice(q * 512, (q + 1) * 512)
            oacc = oap.tile([DH + 1, 1024], f32, tag="oacc", name="oacc")
            for lk in range(LK):
                if hook is not None:
                    hook(lk)
                ksl = slice(lk * 128, (lk + 1) * 128)
                stp = stp_p.tile([128, 1024], f32, tag="stp", name="stp")
                # S^T for both heads, concurrent on PE row groups 0/64.
                # Top priority: the exp chain is the critical resource and
                # S^T is its only feeder - the greedy list-scheduler must
                # prefer it over AV backlog / normalize / filler work.
                with tc.high_priority():
                    nc.tensor.matmul(
                        stp[:, 0:512], lhsT=KT_t[p][0:64, ksl],
                        rhs=QT_t[p][0:64, qsl], start=True, stop=True,
                        tile_position=(0, 0))
                    nc.tensor.matmul(
                        stp[:, 512:1024], lhsT=KT_t[p][64:128, ksl],
                        rhs=QT_t[p][64:128, qsl], start=True, stop=True,
                        tile_position=(64, 0))
                    a_sb = apool.tile([128, 1024], bf16, tag="a", name="a")
                    nc.scalar.activation(a_sb[:], stp[:], Exp)
                    # O^T (+denominator row DH) accumulated over Lk. Also
                    # top priority: AV frees the a-ring the exp chain
                    # rotates through.
                    nc.tensor.matmul(
                        oacc[:, 0:512], lhsT=V_t[lk][:, 2 * p, :],
                        rhs=a_sb[:, 0:512],
                        start=(lk == 0), stop=(lk == LK - 1))
                    nc.tensor.matmul(
                        oacc[:, 512:1024], lhsT=V_t[lk][:, 2 * p + 1, :],
                        rhs=a_sb[:, 512:1024],
                        start=(lk == 0), stop=(lk == LK - 1))
            # Evict the accumulator so the next pair can reuse PSUM
            # (gates the next pair's AV start via the single-buffer oap
            # ring - keep it at top priority).
            oc = nrm.tile([DH + 1, 1024], f32, tag="oc", name="oc", bufs=3)
            with tc.high_priority():
                # Two half-copies: the next pair's head-0 AV only touches
                # oacc cols 0:512, so sub-range tracking lets it start
                # after the first half instead of the full eviction.
                nc.vector.tensor_copy(oc[:, 0:512], oacc[:, 0:512])
                nc.vector.tensor_copy(oc[:, 512:1024], oacc[:, 512:1024])
            if debug:
                nc.sync.dma_start(out=dbg["oc"][4 * q + p], in_=oc[:])
            # Normalized O^T head-pair write (head 2p+1 lands on
            # partitions 64:128 - a DVE quadrant-crossing 64-lane op).
            # OTP is only consumed by NEXT quarter's composition, so for
            # mid-kernel pairs the whole reciprocal chain is deferred and
            # bounced through DRAM (zero PE/ACT involvement, hides in ~2
            # pairs of slack). The very last pair IS the critical path
            # into the final composition: use the low-latency path there
            # (d-row broadcast by a ones-stationary K=1 matmul + fast
            # approx reciprocal at full lane width).
            if q == NQ - 1 and p == NPAIR - 1:
                with tc.high_priority():
                    for half in range(2):
                        hsl = slice(half * 512, (half + 1) * 512)
                        rps = gen_ps.tile([128, 512], f32, tag="pp",
                                          name="pp")
                        nc.tensor.matmul(
                            rps[0:DH, :], lhsT=ones_c[DH:DH + 1, :],
                            rhs=oc[DH:DH + 1, hsl], start=True, stop=True)
                        rcp = nrm.tile([DH, 512], f32, tag="rcp",
                                       name="rcp", bufs=2)
                        nc.vector.reciprocal_approx_fast(out=rcp[:],
                                                         in_=rps[0:DH, :])
                        dst = (OTP_t[p][0:DH, qsl] if half == 0
                               else OTP_t[p][DH:128, qsl])
                        nc.vector.tensor_mul(dst, oc[0:DH, hsl], rcp[:])
            else:
                with tc.high_priority(offset=-250):
                    dn = dscr.tile([1, 1024], f32, tag="dn", name="dn")
                    nc.sync.dma_start(out=dn[:], in_=oc[DH:DH + 1, :])
                    db = nrm.tile([DH, 16], f32, tag="db", name="db")
                    nc.sync.dma_start(
                        out=db[:],
                        in_=dn[:].rearrange("o (p j) -> (o p) j", j=16))
                    rb = nrm.tile([DH, 16], f32, tag="rb", name="rb")
                    nc.vector.reciprocal(out=rb[:], in_=db[:])
                    rd = dscr.tile([1, 1024], f32, tag="rd", name="rd")
                    nc.sync.dma_start(
                        out=rd[:].rearrange("o (p j) -> (o p) j", j=16),
                        in_=rb[:])
                    rcpb = nrm.tile([DH, 1024], f32, tag="rcpb",
                                    name="rcpb")
                    nc.sync.dma_start(out=rcpb[:],
                                      in_=rd[:].to_broadcast([DH, 1024]))
                    nc.vector.tensor_mul(
                        OTP_t[p][0:DH, qsl], oc[0:DH, 0:512],
                        rcpb[:, 0:512])
                    nc.vector.tensor_mul(
                        OTP_t[p][DH:128, qsl], oc[0:DH, 512:1024],
                        rcpb[:, 512:1024])

        if debug:
            for p in range(NPAIR):
                nc.sync.dma_start(out=dbg["qt"][p], in_=QT_t[p][:])
                nc.sync.dma_start(out=dbg["kt"][p], in_=KT_t[p][:])
                nc.sync.dma_start(out=dbg["ot"][p], in_=OTP_t[p][:])
            for m in range(LK):
                nc.sync.dma_start(out=dbg["vt"][m], in_=V_t[m][:])

        # ---- DMA emission in first-need order (one gpsimd-queue FIFO) ----
        wk_t = load_w(wkT, "wk")
        xk_n = [None] * NQ
        xk_n[0] = load_xround("xk0", xkT, 0, bufs=1)
        wq_t = load_w(wqT, "wq")
        xq_rounds = [None] * NQ
        xq_rounds[0] = load_xround("xq", xqT, 0)
        wv_t = load_w(wvT, "wv")
        xv_r = [None] * NQ
        xv_r[0] = load_xround("xv", xvT, 0)
        for n in range(1, NQ):
            xk_n[n] = load_xround(f"xk{n}", xkT, n, bufs=1)
            xv_r[n] = load_xround("xv", xvT, n)
        nc.gpsimd.dma_start(out=wc_all[:], in_=wcT[:])
        if NQ > 1:
            xq_rounds[1] = load_xround("xq", xqT, 1)

        # ---- Lead-in projections: only what gates the first S^T ----
        proj_qk_round(0, wk_t, xk_n[0], KT_t, 0)
        proj_qk_round(0, wq_t, xq_rounds[0], QT_t, 0)

        # ---- Quarter loop with filler hooks ----
        # Fillers are emitted INSIDE the attention chunk loop (producer
        # before consumer, priority between the surrounding chunks) so
        # the list-scheduler drops them into PE slack under the exp-bound
        # attention instead of stalling the exp chain at boundaries:
        #   q0p0: V chunk lk+1 at slot lk, K pair 1 rounds at 3/7/11/15
        #   q0p1: K pair 2 rounds          q0p2: K pair 3 rounds + Q1 x2
        #   q0p3: Q1 x2
        #   q>0:  comp(q-1) chunk halves at p's slots 2/6, Q(q+1) rounds
        #         at pairs 2/3 slots 10/14
        osb = {}

        def comp_filler(m, half):
            def f():
                if half == 0:
                    osb[m] = ost.tile([128, D], f32, tag="osb", name="osb")
                comp_half(m, half, osb[m])
                if half == 1:
                    nc.sync.dma_start(
                        out=outp[m * 128:(m + 1) * 128, :], in_=osb[m][:])
            return f

        def qk_filler(n, w_t, x_n, dst, p):
            return lambda: proj_qk_round(n, w_t, x_n, dst, p)

        def mk_hook(p, q):
            fillers = []
            if q == 0 and p == 0:
                # V chunk lk+1 at slot lk; K pair0 round i before chunk
                # 4i; Q round0 / K pair1 rounds for the later pairs.
                for lk in range(LK - 1):
                    fillers.append(
                        (lk, lambda m=lk + 1: proj_v_chunk(m, xv_r[m // 4],
                                                           wv_t)))
                for i in range(1, NQ):
                    fillers.append(
                        (4 * (i - 1) + 1, qk_filler(i, wk_t, xk_n[i],
                                                    KT_t, 0)))
                for j in range(1, NPAIR):
                    fillers.append(
                        (4 * j - 1, qk_filler(0, wq_t, xq_rounds[0],
                                              QT_t, j)))
                for i in range(NQ):
                    fillers.append(
                        (4 * i + 2, qk_filler(i, wk_t, xk_n[i], KT_t, 1)))
            elif q == 0:
                if p + 1 < NPAIR:
                    for i in range(NQ):
                        fillers.append(
                            (2 + 4 * i,
                             qk_filler(i, wk_t, xk_n[i], KT_t, p + 1)))
                if p == 3 and NQ > 1:
                    fillers.append((2, qk_filler(1, wq_t, xq_rounds[1],
                                                 QT_t, 0)))
            else:
                m = 4 * (q - 1) + p
                # Just-in-time Q rounds FIRST (slot 2): they gate the
                # next pair's / next quarter's S^T; comp(q-1) has a whole
                # quarter of slack, so it yields (slots 6/10).
                if p + 1 < NPAIR:
                    fillers.append(
                        (2, qk_filler(q, wq_t, xq_rounds[q], QT_t, p + 1)))
                elif q + 1 < NQ:
                    fillers.append(
                        (2, qk_filler(q + 1, wq_t, xq_rounds[q + 1],
                                      QT_t, 0)))
                fillers.append((6, comp_filler(m, 0)))
                fillers.append((10, comp_filler(m, 1)))
            fmap = {}
            for slot, f in fillers:
                fmap.setdefault(slot, []).append(f)

            def hook(lk):
                for f in fmap.get(lk, []):
                    f()
            return hook

        # V chunk 0 gates the first AV: emit before the quarter loop.
        proj_v_chunk(0, xv_r[0], wv_t)

        for q in range(NQ):
            if q >= 1 and q + 1 < NQ:
                # Next quarter's X^T round (gpsimd queue is drained).
                xq_rounds[q + 1] = load_xround("xq", xqT, q + 1)
            for p in range(NPAIR):
                attention_quarter(p, q, hook=mk_hook(p, q))

        # Final quarter's composition (tail).
        for m in range(4 * (NQ - 1), 4 * NQ):
            o_sb = ost.tile([128, D], f32, tag="osb", name="osb")
            for half in range(2):
                comp_half(m, half, o_sb)
            nc.sync.dma_start(out=outp[m * 128:(m + 1) * 128, :], in_=o_sb[:])

    nc.compile()
    return nc


def shard_inputs(keys, queries, values, Wk, Wq, Wv, Wc, seq=L):
    """Host-side shard prep: per-core transposed bf16 operands."""

    def bf(a):
        return np.ascontiguousarray(a).astype(ml_dtypes.bfloat16)

    def bft(x):
        # [seq, D] -> X^T per-round contiguous blocks [n, 128, kd*512]
        xt = np.ascontiguousarray(x.T).astype(ml_dtypes.bfloat16)
        kd, nq = xt.shape[0] // 128, xt.shape[1] // 512
        return np.ascontiguousarray(
            xt.reshape(kd, 128, nq, 512).transpose(2, 1, 0, 3)
        ).reshape(nq, 128, kd * 512)

    def wblk(w):
        # [D, G] -> [128, (D//128)*G] with k-block at cols k*G:(k+1)*G
        return bf(np.ascontiguousarray(
            np.asarray(w).reshape(D // 128, 128, G).transpose(1, 0, 2)
        ).reshape(128, (D // 128) * G))

    scale = 1.0 / math.sqrt(DH)
    in_maps = []
    for c in range(N_CORES):
        b, g = c // 2, c % 2
        gs = slice(g * G, (g + 1) * G)
        in_maps.append({
            "xqT": bft(queries[b, :seq]),
            "xkT": bft(keys[b, :seq]),
            "xvT": bft(values[b, :seq]),
            "wqT": wblk(Wq[gs, :].T * scale),
            "wkT": wblk(Wk[gs, :].T),
            "wvT": wblk(Wv[gs, :].T),
            "wcT": bf(np.ascontiguousarray(
                np.asarray(Wc[:, gs].T).reshape(G // 128, 128, D)
                .transpose(1, 0, 2)).reshape(128, (G // 128) * D)),
        })
    return in_maps


_NC_CACHE = {}


def run_cores(inputs, seq=L, trace=False):
    if seq not in _NC_CACHE:
        _NC_CACHE[seq] = build_nc(seq)
    nc = _NC_CACHE[seq]
    in_maps = shard_inputs(
        inputs["keys"], inputs["queries"], inputs["values"],
        inputs["Wk"], inputs["Wq"], inputs["Wv"], inputs["Wc"], seq=seq)
    res = run_bass_kernel_spmd(nc, in_maps, core_ids=list(range(N_CORES)),
                               trace=trace)
    return res


def kernel(keys, queries, values, Wk, Wq, Wv, Wc, bc, attn_mask):
    res = run_cores(dict(keys=np.asarray(keys), queries=np.asarray(queries),
                         values=np.asarray(values), Wk=np.asarray(Wk),
                         Wq=np.asarray(Wq), Wv=np.asarray(Wv),
                         Wc=np.asarray(Wc)))
    bc = np.asarray(bc, np.float32)
    out = np.empty((B, L, D), np.float32)
    for b in range(B):
        out[b] = res.results[2 * b]["outp"] + res.results[2 * b + 1]["outp"] + bc
    return out


# revision 42
# speedup vs baseline: 1.1939x; 1.1939x over previous
"""Distributed dot-product attention for TRN2, 8 NeuronCores.

Sharding: 8 cores = 4 batches x 2 head-groups (8 heads each).
Each core computes, for its (batch b, head-group g):
    Q = Xq[b] @ (Wq[g]/8).T ; K = Xk[b] @ Wk[g].T ; V = Xv[b] @ Wv[g].T
    per head h: A = exp(Q_h K_h^T); O_h = (A V_h) / rowsum(A)
    partial[b,g] = concat_h(O_h) @ Wc[:, g].T            (row-parallel)
Host: out[b] = partial[b,0] + partial[b,1] + bc          (all-reduce + bias)

Device-side layouts avoid every transpose: the host ships X^T and W^T in
bf16; S is computed transposed ([Lk, Lq]) so exp(S^T) feeds AV directly as
the moving operand; AV's stationary V carries a ones column so the softmax
denominators fall out of the same accumulation; composition consumes O^T
HEAD-PAIR blocks [128, m] (head 2p on partitions 0:64, head 2p+1 on
64:128 via a DVE quadrant-crossing write) so each comp matmul contracts
the full 128 rows; it emits the natural-layout f32 partial.

Softmax denominators: mid-kernel pairs bounce the evicted d-row through
DRAM (reshaped to [64, 16]) for a full-lane reciprocal - zero PE/ACT
involvement, deferred ~2 pairs so it hides completely; only the final
pair (the critical path into the last composition) broadcasts the d-row
with a ones-stationary K=1 matmul + reciprocal_approx_fast instead.

Schedule: the exp chain is the critical resource (ACT is the only engine
with the activation LUT; 33.5M exps/core ~ 280us busy). Input rounds load
as single 1MB DMAs in first-need order (per-trigger SWDGE cost otherwise
caps input bandwidth at ~200GB/s), only K pair 0 + Q round 0 gate the
first S^T, and all other projections / previous-quarter composition are
emitted as fine-grained fillers INSIDE the attention chunk loops so the
Tile list-scheduler drops them into PE slack under the exp-bound quarters
instead of stalling the exp chain at pair/quarter boundaries.
"""

import math
from contextlib import ExitStack

import numpy as np
import ml_dtypes

import concourse.bass as bass
import concourse.bacc as bacc
import concourse.tile as tile
from concourse import mybir
from concourse.bass_utils import run_bass_kernel_spmd

B, L, D, H = 4, 2048, 1024, 16
DH = D // H          # 64 per-head dim
HPC = H // 2         # 8 heads per core
G = HPC * DH         # 512 head-group width
N_CORES = 8

f32 = mybir.dt.float32
bf16 = mybir.dt.bfloat16


def build_nc(seq=L, debug=False):
    """Build the per-core Bass program (SPMD, identical on all cores)."""
    KD = D // 128        # 8 contraction chunks over model dim
    LK = seq // 128      # Lk chunks (16)
    NPAIR = HPC // 2     # 4 head-pairs
    NQ = seq // 512      # Lq quarters (4)

    nc = bacc.Bacc(None, target_bir_lowering=False, debug=False)

    # X^T inputs arrive as per-round contiguous 1MB blocks
    # [n, 128, KD*512] (k-major columns): one contiguous DMA per
    # 512-column round (per-trigger SWDGE cost makes eight 128KB loads
    # ~4x slower to issue than one 1MB load).
    xqT = nc.dram_tensor("xqT", [NQ, 128, KD * 512], bf16, kind="ExternalInput")
    xkT = nc.dram_tensor("xkT", [NQ, 128, KD * 512], bf16, kind="ExternalInput")
    xvT = nc.dram_tensor("xvT", [NQ, 128, KD * 512], bf16, kind="ExternalInput")
    wqT = nc.dram_tensor("wqT", [128, KD * G], bf16, kind="ExternalInput")
    wkT = nc.dram_tensor("wkT", [128, KD * G], bf16, kind="ExternalInput")
    wvT = nc.dram_tensor("wvT", [128, KD * G], bf16, kind="ExternalInput")
    wcT = nc.dram_tensor("wcT", [128, (G // 128) * D], bf16,
                         kind="ExternalInput")
    outp = nc.dram_tensor("outp", [seq, D], f32, kind="ExternalOutput")
    dbg = {}
    if debug:
        for nm, shp in [("qt", [4, 128, seq]), ("kt", [4, 128, seq]),
                        ("vt", [16, 128, 8, 65]), ("ot", [4, 128, seq]),
                        ("oc", [16, DH + 1, 1024])]:
            dbg[nm] = nc.dram_tensor(f"dbg_{nm}", shp, f32 if nm == "oc" else bf16,
                                     kind="ExternalOutput")

    with tile.TileContext(nc) as tc, ExitStack() as ctx:
        Exp = mybir.ActivationFunctionType.Exp

        # Persistent SBUF: projected Q^T/K^T (pair tiles: head 2p on
        # partitions 0:64, head 2p+1 on 64:128), V with ones column,
        # normalized O^T as head-pair tiles, composition weight pairs.
        const = ctx.enter_context(tc.tile_pool(name="const", bufs=1))
        QT_t = [const.tile([128, seq], bf16, tag=f"qt{p}", name=f"qt{p}")
                for p in range(NPAIR)]
        KT_t = [const.tile([128, seq], bf16, tag=f"kt{p}", name=f"kt{p}")
                for p in range(NPAIR)]
        V_t = [const.tile([128, HPC, DH + 1], bf16, tag=f"v{m}", name=f"v{m}")
               for m in range(LK)]
        OTP_t = [const.tile([128, seq], bf16, tag=f"otp{p}", name=f"otp{p}")
                 for p in range(NPAIR)]
        wc_all = const.tile([128, NPAIR * D], bf16, tag="wc", name="wc")
        wcP_t = [wc_all[:, p * D:(p + 1) * D] for p in range(NPAIR)]
        ones_c = const.tile([128, DH], f32, tag="ones", name="ones")
        nc.vector.memset(ones_c[:], 1.0)

        wpool = ctx.enter_context(tc.tile_pool(name="wpool", bufs=1))

        def load_w(src, pfx):
            w_all = wpool.tile([128, KD * G], bf16, tag=pfx, name=pfx, bufs=1)
            nc.gpsimd.dma_start(out=w_all[:], in_=src[:])
            return [w_all[:, k * G:(k + 1) * G] for k in range(KD)]

        xcol = ctx.enter_context(tc.tile_pool(name="xcol", bufs=2))

        def load_xround(tag, src, n, bufs=None):
            """One 1MB DMA: round n of X^T -> [128, KD*512] (k-major cols)."""
            t = xcol.tile([128, KD * 512], bf16, tag=tag, name=f"{tag}_{n}",
                          bufs=bufs)
            nc.gpsimd.dma_start(out=t[:], in_=src[n])
            return [t[:, k * 512:(k + 1) * 512] for k in range(KD)]

        # PSUM: projections/composition/denominator-broadcast pp tiles
        # (2 banks), S^T pair tiles (4 banks), AV accumulator (2) = 8.
        gen_ps = ctx.enter_context(
            tc.tile_pool(name="gen_ps", bufs=2, space=bass.MemorySpace.PSUM))
        stp_p = ctx.enter_context(
            tc.tile_pool(name="stp", bufs=2, space=bass.MemorySpace.PSUM))
        oap = ctx.enter_context(
            tc.tile_pool(name="oap", bufs=1, space=bass.MemorySpace.PSUM))
        apool = ctx.enter_context(tc.tile_pool(name="apool", bufs=2))
        nrm = ctx.enter_context(tc.tile_pool(name="nrm", bufs=2))
        ost = ctx.enter_context(tc.tile_pool(name="ost", bufs=3))
        dscr = ctx.enter_context(
            tc.tile_pool(name="dscr", bufs=2, space=bass.MemorySpace.DRAM))

        def proj_qk_round(n, w_t, x_n, dst, p):
            """dst[p][:, n-cols] = (W block p).T @ X^T[:, n-cols], bf16."""
            ps = gen_ps.tile([128, 512], f32, tag="pp", name="pp")
            for k in range(KD):
                nc.tensor.matmul(
                    ps[:], lhsT=w_t[k][:, p * 128:(p + 1) * 128],
                    rhs=x_n[k], start=(k == 0), stop=(k == KD - 1))
            nc.vector.tensor_copy(dst[p][:, n * 512:(n + 1) * 512], ps[:])

        def proj_v_chunk(m, xv_n, wv_t):
            """V_t[m] (one 128-row chunk of V, with ones column)."""
            mm = m % 4
            ps = gen_ps.tile([128, G], f32, tag="pp", name="pp")
            for k in range(KD):
                nc.tensor.matmul(
                    ps[:], lhsT=xv_n[k][:, mm * 128:(mm + 1) * 128],
                    rhs=wv_t[k], start=(k == 0), stop=(k == KD - 1))
            nc.vector.tensor_copy(
                V_t[m][:, :, 0:DH], ps[:].rearrange("p (h d) -> p h d", h=HPC))
            nc.vector.memset(V_t[m][:, :, DH:DH + 1], 1.0)

        def comp_half(m, half, o_sb):
            """One half of composition output row-chunk m."""
            msl = slice(m * 128, (m + 1) * 128)
            hsl = slice(half * 512, (half + 1) * 512)
            ps = gen_ps.tile([128, 512], f32, tag="pp", name="pp")
            for p in range(NPAIR):
                nc.tensor.matmul(
                    ps[:], lhsT=OTP_t[p][:, msl], rhs=wcP_t[p][:, hsl],
                    start=(p == 0), stop=(p == NPAIR - 1))
            nc.vector.tensor_copy(o_sb[:, hsl], ps[:])

        def attention_quarter(p, q, hook=None):
            qsl = slice(q * 512, (q + 1) * 512)
            oacc = oap.tile([DH + 1, 1024], f32, tag="oacc", name="oacc")
            for lk in range(LK):
                if hook is not None:
                    hook(lk)
                ksl = slice(lk * 128, (lk + 1) * 128)
                stp = stp_p.tile([128, 1024], f32, tag="stp", name="stp")
                # S^T for both heads, concurrent on PE row groups 0/64.
                # Top priority: the exp chain is the critical resource and
                # S^T is its only feeder - the greedy list-scheduler must
                # prefer it over AV backlog / normalize / filler work.
                with tc.high_priority():
                    nc.tensor.matmul(
                        stp[:, 0:512], lhsT=KT_t[p][0:64, ksl],
                        rhs=QT_t[p][0:64, qsl], start=True, stop=True,
                        tile_position=(0, 0))
                    nc.tensor.matmul(
                        stp[:, 512:1024], lhsT=KT_t[p][64:128, ksl],
                        rhs=QT_t[p][64:128, qsl], start=True, stop=True,
                        tile_position=(64, 0))
                    a_sb = apool.tile([128, 1024], bf16, tag="a", name="a")
                    nc.scalar.activation(a_sb[:], stp[:], Exp)
                    # O^T (+denominator row DH) accumulated over Lk. Also
                    # top priority: AV frees the a-ring the exp chain
                    # rotates through.
                    nc.tensor.matmul(
                        oacc[:, 0:512], lhsT=V_t[lk][:, 2 * p, :],
                        rhs=a_sb[:, 0:512],
                        start=(lk == 0), stop=(lk == LK - 1))
                    nc.tensor.matmul(
                        oacc[:, 512:1024], lhsT=V_t[lk][:, 2 * p + 1, :],
                        rhs=a_sb[:, 512:1024],
                        start=(lk == 0), stop=(lk == LK - 1))
            # Evict the accumulator so the next pair can reuse PSUM
            # (gates the next pair's AV start via the single-buffer oap
            # ring - keep it at top priority).
            oc = nrm.tile([DH + 1, 1024], f32, tag="oc", name="oc", bufs=3)
            with tc.high_priority():
                nc.vector.tensor_copy(oc[:], oacc[:])
            if debug:
                nc.sync.dma_start(out=dbg["oc"][4 * q + p], in_=oc[:])
            # Normalized O^T head-pair write (head 2p+1 lands on
            # partitions 64:128 - a DVE quadrant-crossing 64-lane op).
            # OTP is only consumed by NEXT quarter's composition, so for
            # mid-kernel pairs the whole reciprocal chain is deferred and
            # bounced through DRAM (zero PE/ACT involvement, hides in ~2
            # pairs of slack). The very last pair IS the critical path
            # into the final composition: use the low-latency path there
            # (d-row broadcast by a ones-stationary K=1 matmul + fast
            # approx reciprocal at full lane width).
            if q == NQ - 1 and p == NPAIR - 1:
                with tc.high_priority():
                    for half in range(2):
                        hsl = slice(half * 512, (half + 1) * 512)
                        rps = gen_ps.tile([128, 512], f32, tag="pp",
                                          name="pp")
                        nc.tensor.matmul(
                            rps[0:DH, :], lhsT=ones_c[DH:DH + 1, :],
                            rhs=oc[DH:DH + 1, hsl], start=True, stop=True)
                        rcp = nrm.tile([DH, 512], f32, tag="rcp",
                                       name="rcp", bufs=2)
                        nc.vector.reciprocal_approx_fast(out=rcp[:],
                                                         in_=rps[0:DH, :])
                        dst = (OTP_t[p][0:DH, qsl] if half == 0
                               else OTP_t[p][DH:128, qsl])
                        nc.vector.tensor_mul(dst, oc[0:DH, hsl], rcp[:])
            else:
                with tc.high_priority(offset=-250):
                    dn = dscr.tile([1, 1024], f32, tag="dn", name="dn")
                    nc.sync.dma_start(out=dn[:], in_=oc[DH:DH + 1, :])
                    db = nrm.tile([DH, 16], f32, tag="db", name="db")
                    nc.sync.dma_start(
                        out=db[:],
                        in_=dn[:].rearrange("o (p j) -> (o p) j", j=16))
                    rb = nrm.tile([DH, 16], f32, tag="rb", name="rb")
                    nc.vector.reciprocal(out=rb[:], in_=db[:])
                    rd = dscr.tile([1, 1024], f32, tag="rd", name="rd")
                    nc.sync.dma_start(
                        out=rd[:].rearrange("o (p j) -> (o p) j", j=16),
                        in_=rb[:])
                    rcpb = nrm.tile([DH, 1024], f32, tag="rcpb",
                                    name="rcpb")
                    nc.sync.dma_start(out=rcpb[:],
                                      in_=rd[:].to_broadcast([DH, 1024]))
                    nc.vector.tensor_mul(
                        OTP_t[p][0:DH, qsl], oc[0:DH, 0:512],
                        rcpb[:, 0:512])
                    nc.vector.tensor_mul(
                        OTP_t[p][DH:128, qsl], oc[0:DH, 512:1024],
                        rcpb[:, 512:1024])

        if debug:
            for p in range(NPAIR):
                nc.sync.dma_start(out=dbg["qt"][p], in_=QT_t[p][:])
                nc.sync.dma_start(out=dbg["kt"][p], in_=KT_t[p][:])
                nc.sync.dma_start(out=dbg["ot"][p], in_=OTP_t[p][:])
            for m in range(LK):
                nc.sync.dma_start(out=dbg["vt"][m], in_=V_t[m][:])

        # ---- DMA emission in first-need order (one gpsimd-queue FIFO) ----
        wk_t = load_w(wkT, "wk")
        xk_n = [None] * NQ
        xk_n[0] = load_xround("xk0", xkT, 0, bufs=1)
        wq_t = load_w(wqT, "wq")
        xq_rounds = [None] * NQ
        xq_rounds[0] = load_xround("xq", xqT, 0)
        wv_t = load_w(wvT, "wv")
        xv_r = [None] * NQ
        xv_r[0] = load_xround("xv", xvT, 0)
        for n in range(1, NQ):
            xk_n[n] = load_xround(f"xk{n}", xkT, n, bufs=1)
            xv_r[n] = load_xround("xv", xvT, n)
        nc.gpsimd.dma_start(out=wc_all[:], in_=wcT[:])
        if NQ > 1:
            xq_rounds[1] = load_xround("xq", xqT, 1)

        # ---- Lead-in projections: only what gates the first S^T ----
        proj_qk_round(0, wk_t, xk_n[0], KT_t, 0)
        proj_qk_round(0, wq_t, xq_rounds[0], QT_t, 0)

        # ---- Quarter loop with filler hooks ----
        # Fillers are emitted INSIDE the attention chunk loop (producer
        # before consumer, priority between the surrounding chunks) so
        # the list-scheduler drops them into PE slack under the exp-bound
        # attention instead of stalling the exp chain at boundaries:
        #   q0p0: V chunk lk+1 at slot lk, K pair 1 rounds at 3/7/11/15
        #   q0p1: K pair 2 rounds          q0p2: K pair 3 rounds + Q1 x2
        #   q0p3: Q1 x2
        #   q>0:  comp(q-1) chunk halves at p's slots 2/6, Q(q+1) rounds
        #         at pairs 2/3 slots 10/14
        osb = {}

        def comp_filler(m, half):
            def f():
                if half == 0:
                    osb[m] = ost.tile([128, D], f32, tag="osb", name="osb")
                comp_half(m, half, osb[m])
                if half == 1:
                    nc.sync.dma_start(
                        out=outp[m * 128:(m + 1) * 128, :], in_=osb[m][:])
            return f

        def qk_filler(n, w_t, x_n, dst, p):
            return lambda: proj_qk_round(n, w_t, x_n, dst, p)

        def mk_hook(p, q):
            fillers = []
            if q == 0 and p == 0:
                # V chunk lk+1 at slot lk; K pair0 round i before chunk
                # 4i; Q round0 / K pair1 rounds for the later pairs.
                for lk in range(LK - 1):
                    fillers.append(
                        (lk, lambda m=lk + 1: proj_v_chunk(m, xv_r[m // 4],
                                                           wv_t)))
                for i in range(1, NQ):
                    fillers.append(
                        (4 * (i - 1) + 1, qk_filler(i, wk_t, xk_n[i],
                                                    KT_t, 0)))
                for j in range(1, NPAIR):
                    fillers.append(
                        (4 * j - 1, qk_filler(0, wq_t, xq_rounds[0],
                                              QT_t, j)))
                for i in range(NQ):
                    fillers.append(
                        (4 * i + 2, qk_filler(i, wk_t, xk_n[i], KT_t, 1)))
            elif q == 0:
                if p + 1 < NPAIR:
                    for i in range(NQ):
                        fillers.append(
                            (2 + 4 * i,
                             qk_filler(i, wk_t, xk_n[i], KT_t, p + 1)))
                if p == 3 and NQ > 1:
                    fillers.append((2, qk_filler(1, wq_t, xq_rounds[1],
                                                 QT_t, 0)))
            else:
                m = 4 * (q - 1) + p
                # Just-in-time Q rounds FIRST (slot 2): they gate the
                # next pair's / next quarter's S^T; comp(q-1) has a whole
                # quarter of slack, so it yields (slots 6/10).
                if p + 1 < NPAIR:
                    fillers.append(
                        (2, qk_filler(q, wq_t, xq_rounds[q], QT_t, p + 1)))
                elif q + 1 < NQ:
                    fillers.append(
                        (2, qk_filler(q + 1, wq_t, xq_rounds[q + 1],
                                      QT_t, 0)))
                fillers.append((6, comp_filler(m, 0)))
                fillers.append((10, comp_filler(m, 1)))
            fmap = {}
            for slot, f in fillers:
                fmap.setdefault(slot, []).append(f)

            def hook(lk):
                for f in fmap.get(lk, []):
                    f()
            return hook

        # V chunk 0 gates the first AV: emit before the quarter loop.
        proj_v_chunk(0, xv_r[0], wv_t)

        for q in range(NQ):
            if q >= 1 and q + 1 < NQ:
                # Next quarter's X^T round (gpsimd queue is drained).
                xq_rounds[q + 1] = load_xround("xq", xqT, q + 1)
            for p in range(NPAIR):
                attention_quarter(p, q, hook=mk_hook(p, q))

        # Final quarter's composition (tail).
        for m in range(4 * (NQ - 1), 4 * NQ):
            o_sb = ost.tile([128, D], f32, tag="osb", name="osb")
            for half in range(2):
                comp_half(m, half, o_sb)
            nc.sync.dma_start(out=outp[m * 128:(m + 1) * 128, :], in_=o_sb[:])

    nc.compile()
    return nc


def shard_inputs(keys, queries, values, Wk, Wq, Wv, Wc, seq=L):
    """Host-side shard prep: per-core transposed bf16 operands."""

    def bf(a):
        return np.ascontiguousarray(a).astype(ml_dtypes.bfloat16)

    def bft(x):
        # [seq, D] -> X^T per-round contiguous blocks [n, 128, kd*512]
        xt = np.ascontiguousarray(x.T).astype(ml_dtypes.bfloat16)
        kd, nq = xt.shape[0] // 128, xt.shape[1] // 512
        return np.ascontiguousarray(
            xt.reshape(kd, 128, nq, 512).transpose(2, 1, 0, 3)
        ).reshape(nq, 128, kd * 512)

    def wblk(w):
        # [D, G] -> [128, (D//128)*G] with k-block at cols k*G:(k+1)*G
        return bf(np.ascontiguousarray(
            np.asarray(w).reshape(D // 128, 128, G).transpose(1, 0, 2)
        ).reshape(128, (D // 128) * G))

    scale = 1.0 / math.sqrt(DH)
    in_maps = []
    for c in range(N_CORES):
        b, g = c // 2, c % 2
        gs = slice(g * G, (g + 1) * G)
        in_maps.append({
            "xqT": bft(queries[b, :seq]),
            "xkT": bft(keys[b, :seq]),
            "xvT": bft(values[b, :seq]),
            "wqT": wblk(Wq[gs, :].T * scale),
            "wkT": wblk(Wk[gs, :].T),
            "wvT": wblk(Wv[gs, :].T),
            "wcT": bf(np.ascontiguousarray(
                np.asarray(Wc[:, gs].T).reshape(G // 128, 128, D)
                .transpose(1, 0, 2)).reshape(128, (G // 128) * D)),
        })
    return in_maps


_NC_CACHE = {}


def run_cores(inputs, seq=L, trace=False):
    if seq not in _NC_CACHE:
        _NC_CACHE[seq] = build_nc(seq)
    nc = _NC_CACHE[seq]
    in_maps = shard_inputs(
        inputs["keys"], inputs["queries"], inputs["values"],
        inputs["Wk"], inputs["Wq"], inputs["Wv"], inputs["Wc"], seq=seq)
    res = run_bass_kernel_spmd(nc, in_maps, core_ids=list(range(N_CORES)),
                               trace=trace)
    return res


def kernel(keys, queries, values, Wk, Wq, Wv, Wc, bc, attn_mask):
    res = run_cores(dict(keys=np.asarray(keys), queries=np.asarray(queries),
                         values=np.asarray(values), Wk=np.asarray(Wk),
                         Wq=np.asarray(Wq), Wv=np.asarray(Wv),
                         Wc=np.asarray(Wc)))
    bc = np.asarray(bc, np.float32)
    out = np.empty((B, L, D), np.float32)
    for b in range(B):
        out[b] = res.results[2 * b]["outp"] + res.results[2 * b + 1]["outp"] + bc
    return out
